# revision 48
# baseline (speedup 1.0000x reference)
"""BSRoformer attention kernel for 8 TRN2 NeuronCores (v3).

Sharding: 8 cores = 4 batch elements x 2 tensor-parallel halves.
Core c handles batch b = c//2, TP half t = c%2 (q heads 8t..8t+8,
kv heads 2t..2t+2, Wo columns 512t..512t+512). Host sums the two
row-parallel O-projection partials per batch element.

v3 (vs v2): the softmax exp is split across engines -- ACT exps 12 of
16 k-tiles, DVE computes k-tiles 8..11 with a bitcast fast-exp
(int16(round(x*c1+c2)) viewed as bf16 == 2^y with linear mantissa
interpolation; c2 is centered so the approximation is mean-unbiased and
softmax normalization cancels the residual ~2% ripple). Scores flow
through a 3-deep ring of 2-bank PSUM groups (the two DVE groups sit
mid-pos so the ring slots the next head needs first are freed early --
with only 2 slots, one dep-waiting score group clogs the PE's 4-deep
wait queue and stalls ready work behind it). PV accumulates all 4
q-subtiles into one bank (one strided reciprocal per head), the
anp->at transposes ride the XBAR DMA engine (one 3-D dma_transpose per
pair; PE transposes only at the drain where the DMA round-trip would
sit on the tail), each weight matrix lands in SBUF via one DMA with
the x/wq loads staged in chase order, O-projection output is bf16
(host upcasts), and next-chunk Q-projections spill past the chunk
boundary to balance chunk-0's K/V-proj load.
"""

import sys
from collections import deque
from contextlib import ExitStack

import numpy as np

try:
    import concourse.bass as bass  # noqa: F401
except Exception:  # pragma: no cover
    sys.path.insert(0, "/opt/trn_rl_repo")
    import concourse.bass as bass  # noqa: F401

import ml_dtypes
import concourse.mybir as mybir
import concourse.tile as tile
from concourse import bacc
from concourse.bass_utils import run_bass_kernel_spmd

F32 = mybir.dt.float32
BF16 = mybir.dt.bfloat16
I16 = mybir.dt.int16

B, S, HID = 4, 2048, 1024
NH, NKV, HD = 16, 4, 64
SCALE = HD ** -0.5
NHL, NKVL = 8, 2               # per-core q heads / kv heads (TP half)
QD, KD = NHL * HD, NKVL * HD   # 512, 128
VD = KD + NKVL                 # 130: [v_g0(64) | ones | v_g1(64) | ones]
NKT = S // 128                 # 16 key tiles
KTG = 2                        # key tiles per psum scores group
NGRP = NKT // KTG              # 8 score groups per (q-chunk, head)
NDVE = 2                       # trailing groups computed by DVE fast-exp
NQC = S // 512                 # 4 query chunks
ALU = mybir.AluOpType

# fast-exp: bf16 bits of 2^(x*log2e) ~= int16(x*FEXP_C1 + FEXP_C2).
# FEXP_C2 is lowered by 128*log2(E[(1+f)2^-f]) = 7.3737 so the linear
# mantissa interpolation is mean-unbiased against the exact-exp tiles.
FEXP_C1 = SCALE * 128.0 * 1.4426950408889634
FEXP_C2 = 127.0 * 128.0 - 7.3737

LAST_RESULTS = None


def build_bass(with_bias=False):
    nc = bacc.Bacc()
    inp = {}
    decls = [
        ("xT", [HID, S], BF16),
        ("wkT", [128, 8 * KD], BF16),    # c-folded: [r, c*128+col]
        ("wqT", [128, 8 * QD], BF16),    # c-folded: [r, c*512+m]
        ("wvT", [128, 8 * VD], BF16),    # c-folded: [r, c*130+col]
        ("wvB", [1, VD], BF16),          # bias row + ones columns
        ("woT", [128, 4 * HID], BF16),   # pair-folded: [r, pp*1024+col]
        ("ident", [128, 128], BF16),
        ("cosT2", [128, S], BF16),
        ("ssinT2", [128, S], BF16),
        ("rmat", [128, 128], BF16),
    ]
    if with_bias:
        decls += [("wqB", [1, QD], BF16), ("wkB", [1, KD], BF16)]
    for name, shape, dt in decls:
        inp[name] = nc.declare_dram_parameter(name, shape, dt, isOutput=False).ap()
    out = nc.declare_dram_parameter("out", [S, HID], BF16, isOutput=True).ap()

    with ExitStack() as ctx:
        tc = ctx.enter_context(tile.TileContext(nc))
        consts = ctx.enter_context(tc.tile_pool(name="consts", bufs=1))
        qtp = ctx.enter_context(tc.tile_pool(name="qtp", bufs=1))
        ktp = ctx.enter_context(tc.tile_pool(name="ktp", bufs=1))
        vp = ctx.enter_context(tc.tile_pool(name="vp", bufs=1))
        rpp = ctx.enter_context(tc.tile_pool(name="rpp", bufs=6))
        tsp = ctx.enter_context(tc.tile_pool(name="tsp", bufs=6))
        ptp = ctx.enter_context(tc.tile_pool(name="ptp", bufs=33))
        rdp = ctx.enter_context(tc.tile_pool(name="rdp", bufs=8))
        anpool = ctx.enter_context(tc.tile_pool(name="anpool", bufs=8))
        atp = ctx.enter_context(tc.tile_pool(name="atp", bufs=8))
        osp = ctx.enter_context(tc.tile_pool(name="osp", bufs=6))
        pjp = ctx.enter_context(tc.tile_pool(name="pjp", bufs=2, space="PSUM"))
        scp = ctx.enter_context(tc.tile_pool(name="scp", bufs=3, space="PSUM"))

        # ---------------- constants / weights in SBUF ----------------
        # One DMA per weight matrix (c-folded host layouts); x arrives in
        # 4 column phases (phase 0 per-c) so K-proj chases the loads.
        wk_sb = consts.tile([128, 8 * KD], BF16, tag="wk", name="wk")
        nc.sync.dma_start(out=wk_sb, in_=inp["wkT"])
        xt_all = consts.tile([128, 8 * S], BF16, tag="xt", name="xt")
        wq_sb = consts.tile([128, 8 * QD], BF16, tag="wq", name="wq")
        wq_v = wq_sb[:, :].rearrange("p (c m) -> p c m", c=8)
        wqT_v = inp["wqT"][:, :].rearrange("p (c m) -> p c m", c=8)

        def xt(c, lo, hi):
            return xt_all[:, c * S + lo: c * S + hi]

        xt_v = xt_all[:, :].rearrange("p (c s) -> p c s", c=8)
        xT_v = inp["xT"][:, :].rearrange("(c p) s -> p c s", c=8)

        def dma_x_phase(ph, halves=False):
            sl = slice(ph * 512, (ph + 1) * 512)
            if halves:
                nc.sync.dma_start(out=xt_v[:, 0:4, sl], in_=xT_v[:, 0:4, sl])
                nc.sync.dma_start(out=xt_v[:, 4:8, sl], in_=xT_v[:, 4:8, sl])
            else:
                nc.sync.dma_start(out=xt_v[:, :, sl], in_=xT_v[:, :, sl])

        # startup critical path: x phase-0 per-c right after wk so the fused
        # K/Q-proj chases the DMAs; the wq pair-0 slice lands right after
        # x c0 (the fused loop needs it at c0), rope tables in halves, and
        # the x phases run ahead of the bulk weight loads
        def dma_xc(c):
            nc.sync.dma_start(out=xt_all[:, c * S: c * S + 512],
                              in_=inp["xT"][c * 128:(c + 1) * 128, 0:512])

        dma_xc(0)
        nc.sync.dma_start(out=wq_v[:, :, 0:128], in_=wqT_v[:, :, 0:128])
        for c in range(1, 4):
            dma_xc(c)
        rmat = consts.tile([128, 128], BF16, tag="rmat", name="rmat")
        nc.sync.dma_start(out=rmat, in_=inp["rmat"])
        cos_sb = consts.tile([128, S], BF16, tag="cos", name="cos")
        sin_sb = consts.tile([128, S], BF16, tag="sin", name="sin")
        nc.sync.dma_start(out=cos_sb[:, 0:512], in_=inp["cosT2"][:, 0:512])
        nc.sync.dma_start(out=sin_sb[:, 0:512], in_=inp["ssinT2"][:, 0:512])
        for c in range(4, 8):
            dma_xc(c)
        dma_x_phase(1, halves=True)
        nc.sync.dma_start(out=wq_v[:, :, 128:256], in_=wqT_v[:, :, 128:256])
        nc.sync.dma_start(out=cos_sb[:, 512:2048], in_=inp["cosT2"][:, 512:2048])
        nc.sync.dma_start(out=sin_sb[:, 512:2048], in_=inp["ssinT2"][:, 512:2048])
        wv_b = consts.tile([1, VD], BF16, tag="wv_b", name="wv_b")
        nc.sync.dma_start(out=wv_b, in_=inp["wvB"])
        dma_x_phase(2, halves=True)
        nc.sync.dma_start(out=wq_v[:, :, 256:512], in_=wqT_v[:, :, 256:512])
        wv_sb = consts.tile([128, 8 * VD], BF16, tag="wv", name="wv")
        nc.sync.dma_start(out=wv_sb, in_=inp["wvT"])
        dma_x_phase(3)
        wo_sb = consts.tile([128, 4 * HID], BF16, tag="wo", name="wo")
        nc.sync.dma_start(out=wo_sb, in_=inp["woT"])
        identT = consts.tile([128, 128], BF16, tag="ident", name="ident")
        nc.sync.dma_start(out=identT, in_=inp["ident"])
        if with_bias:
            wq_b = consts.tile([1, QD], BF16, tag="wq_b", name="wq_b")
            nc.sync.dma_start(out=wq_b, in_=inp["wqB"])
            wk_b = consts.tile([1, KD], BF16, tag="wk_b", name="wk_b")
            nc.sync.dma_start(out=wk_b, in_=inp["wkB"])
        ones_f = consts.tile([1, S], BF16, tag="ones_f", name="ones_f")
        nc.vector.memset(ones_f, 1.0)

        # ---------------- rope ----------------
        # rotate-half runs on the PE (rot_ps = P_swap @ pb); the DVE ops are
        # then all partition-aligned: ts = rot_ps*sgn-sin, dst = pb*cos + ts.
        # rot_ps reuses the projection's own psum bank (its data is already
        # evacuated to pb when the rotate matmul runs).
        def rope_b2(pb, dst, dsl, qs, rot_ps):
            nc.tensor.matmul(rot_ps, rmat, pb, start=True, stop=True)
            ts = tsp.tile([128, 512], BF16, tag="ts", name="ts")
            nc.vector.tensor_mul(ts, rot_ps, sin_sb[:, qs])
            nc.vector.tensor_mul(dst[:, dsl], pb, cos_sb[:, qs])
            nc.vector.tensor_add(dst[:, dsl], dst[:, dsl], ts)

        # ---------------- K projection + rope + head-pair dup ----------------
        kt_raw = ktp.tile([128, S], BF16, tag="kt_raw", name="kt_raw")
        ktd = [ktp.tile([128, S], BF16, tag=f"ktd{g}", name=f"ktd{g}") for g in range(2)]

        def kproj_chunk(q4):
            qs = slice(q4 * 512, (q4 + 1) * 512)
            ps = pjp.tile([128, 512], F32, tag="pj", name="pj")
            for c in range(8):
                nc.tensor.matmul(ps, wk_sb[:, c * KD:(c + 1) * KD],
                                 xt(c, qs.start, qs.stop), start=(c == 0),
                                 stop=(not with_bias and c == 7))
            if with_bias:
                nc.tensor.matmul(ps, wk_b, ones_f[0:1, qs], start=False, stop=True)
            pb = rpp.tile([128, 512], BF16, tag="pb", name="pb")
            nc.vector.tensor_copy(pb, ps)
            rope_b2(pb, kt_raw, qs, qs, ps)
            nc.gpsimd.dma_start(out=ktd[0][64:128, qs], in_=kt_raw[0:64, qs])
            nc.gpsimd.dma_start(out=ktd[1][0:64, qs], in_=kt_raw[64:128, qs])

        # ---------------- Q projection (per chunk, pipelined) ----------------
        qt = [qtp.tile([128, 1024], BF16, tag=f"qt{i}", name=f"qt{i}") for i in range(4)]

        def qproj_group(qc, i):
            qs = slice(qc * 512, (qc + 1) * 512)
            slot = slice((qc % 2) * 512, (qc % 2 + 1) * 512)
            ps = pjp.tile([128, 512], F32, tag="pj", name="pj")
            for c in range(8):
                nc.tensor.matmul(ps, wq_sb[:, c * QD + i * 128: c * QD + (i + 1) * 128],
                                 xt(c, qs.start, qs.stop), start=(c == 0),
                                 stop=(not with_bias and c == 7))
            if with_bias:
                nc.tensor.matmul(ps, wq_b[:, i * 128:(i + 1) * 128],
                                 ones_f[0:1, qs], start=False, stop=True)
            pb = rpp.tile([128, 512], BF16, tag="pb", name="pb")
            nc.vector.tensor_copy(pb, ps)
            rope_b2(pb, qt[i], slot, qs, ps)

        # ---------------- V projection (natural, with ones columns) --------
        vt = [vp.tile([128, VD], BF16, tag=f"v{i}", name=f"v{i}") for i in range(NKT)]

        def emit_vproj(i0, i1):
            for i in range(i0, i1):
                tsl = slice(i * 128, (i + 1) * 128)
                ps = pjp.tile([128, 512], F32, tag="pj", name="pj")
                psv = ps[:, 0:VD]
                for c in range(8):
                    nc.tensor.matmul(psv, xt(c, tsl.start, tsl.stop),
                                     wv_sb[:, c * VD:(c + 1) * VD],
                                     start=(c == 0), stop=False)
                # the wv bias row also carries the ones columns that produce
                # the softmax denominators, so it is always emitted
                nc.tensor.matmul(psv, ones_f[0:1, tsl], wv_b, start=False, stop=True)
                nc.vector.tensor_copy(vt[i], psv)

        # ---------------- attention helpers ----------------
        at_tiles = {}
        anp_of = {}

        def emit_score_group(q, h, gi, pts, on_dve):
            p, ab, g = h // 2, h % 2, h // 4
            qs = slice((q % 2) * 512, (q % 2 + 1) * 512)
            qrow = slice(ab * 64, (ab + 1) * 64)
            kt_src = kt_raw if ab == g else ktd[g]
            sc = scp.tile([128, KTG * 512], F32, tag="sc", name="sc")
            for j in range(KTG):
                kt_i = gi * KTG + j
                ksl = slice(kt_i * 128, (kt_i + 1) * 128)
                nc.tensor.matmul(sc[:, j * 512:(j + 1) * 512],
                                 kt_src[qrow, ksl], qt[p][qrow, qs],
                                 start=True, stop=True)
            pt = pts[gi]
            if on_dve:
                nc.vector.tensor_scalar(pt.bitcast(I16), sc, FEXP_C1, FEXP_C2,
                                        ALU.mult, ALU.add)
            else:
                nc.scalar.activation(out=pt, in_=sc,
                                     func=mybir.ActivationFunctionType.Exp,
                                     scale=SCALE)

        def pv_segment(q, h, pts, pvt, t):
            g = h // 4
            dst = pvt[:, t * 65:(t + 1) * 65]
            for gi in range(NGRP):
                for j in range(KTG):
                    kt_i = gi * KTG + j
                    nc.tensor.matmul(dst, pts[gi][:, j, t * 128:(t + 1) * 128],
                                     vt[kt_i][:, g * 65:(g + 1) * 65],
                                     start=(kt_i == 0), stop=(kt_i == NKT - 1))

        pair_done = {}

        def pv_finalize(q, h, pvt):
            p, ab = h // 2, h % 2
            anp = anp_of[(q, p)]
            rd = rdp.tile([128, 4], F32, tag="rd", name="rd")
            nc.vector.reciprocal(rd, pvt[:, 64:64 + 4 * 65:65])
            for t in range(4):
                base = t * 128 + ab * 64
                nc.vector.tensor_scalar(anp[:, base:base + 64],
                                        pvt[:, t * 65:t * 65 + 64],
                                        rd[:, t:t + 1], None, ALU.mult)
            n = pair_done.get((q, p), 0) + 1
            pair_done[(q, p)] = n
            if n == 2:
                transpose_pair(q, p, range(4), use_pe=drain_mode[0])

        def transpose_pair(q, p, ts_, use_pe=False):
            # one XBAR DMA transposes the whole pair: out[d, t, q] =
            # anp[q, t*128+d] (the per-q-subtile transpose the O-proj
            # needs). The drain uses PE transposes instead: the ~2.8us DMA
            # round-trip would sit on the critical tail path.
            at = at_tiles[q]
            anp = anp_of[(q, p)]
            if use_pe:
                tp_f = pjp.tile([128, 512], F32, tag="pj", name="pj")
                tp_b = tp_f.bitcast(BF16)
                for t in ts_:
                    nc.tensor.transpose(tp_b[:, t * 128:(t + 1) * 128],
                                        anp[:, t * 128:(t + 1) * 128], identT)
                nc.vector.tensor_copy(at[p][:, 0:512], tp_b[:, 0:512])
            else:
                out_v = at[p][:, 0:512].rearrange("d (t q) -> d t q", t=4)
                nc.sync.dma_start_transpose(out=out_v, in_=anp[:, 0:512])

        def oproj_group(q, gidx, drain=False):
            t_, nh = gidx // 2, gidx % 2
            at = at_tiles[q]
            rows = slice(q * 512 + t_ * 128, q * 512 + (t_ + 1) * 128)
            tsl = slice(t_ * 128, (t_ + 1) * 128)
            nsl = slice(nh * 512, (nh + 1) * 512)
            if drain and gidx % 2 == 1:
                # tail: scores banks are free, use them to widen the pipeline
                po = scp.tile([128, KTG * 512], F32, tag="sc", name="sc")[:, 0:512]
            else:
                po = pjp.tile([128, 512], F32, tag="pj", name="pj")
            for pp in range(4):
                nc.tensor.matmul(po, at[pp][:, tsl],
                                 wo_sb[:, pp * HID + nsl.start: pp * HID + nsl.stop],
                                 start=(pp == 0), stop=(pp == 3))
            os_ = osp.tile([128, 512], BF16, tag="os", name="os")
            if drain and gidx % 2 == 0:
                nc.scalar.activation(out=os_, in_=po,
                                     func=mybir.ActivationFunctionType.Copy)
            else:
                nc.vector.tensor_copy(os_, po)
            nc.sync.dma_start(out=out[rows, nsl], in_=os_)

        # ---------------- emission schedule ----------------
        # Scores of head (q,pos) are interleaved with "filler" PE segments
        # (PV of the head popped from the pipeline, O-proj of the previous
        # chunk, Q-proj of the next, chunk-0 K/V-proj) so the PE stream
        # always has work while ACT/DVE chew on exps.
        HORDER = [0, 2, 5, 7, 1, 3, 4, 6]  # dup-free heads (ab==g) first
        pending = deque()
        drain_mode = [False]
        DEPTH_OF = [3, 2, 2, 2]

        # fused K-proj(chunk0) + Q-proj(chunk0, pair0): the two accumulation
        # series interleave per-c so both chase the per-c x DMAs
        def kq_startup():
            qs = slice(0, 512)
            ps_k = pjp.tile([128, 512], F32, tag="pj", name="pj")
            ps_q = pjp.tile([128, 512], F32, tag="pj", name="pj")
            last = not with_bias
            for c in range(8):
                nc.tensor.matmul(ps_k, wk_sb[:, c * KD:(c + 1) * KD],
                                 xt(c, 0, 512), start=(c == 0),
                                 stop=(last and c == 7))
                nc.tensor.matmul(ps_q, wq_sb[:, c * QD:c * QD + 128],
                                 xt(c, 0, 512), start=(c == 0),
                                 stop=(last and c == 7))
            if with_bias:
                nc.tensor.matmul(ps_k, wk_b, ones_f[0:1, qs], start=False, stop=True)
                nc.tensor.matmul(ps_q, wq_b[:, 0:128], ones_f[0:1, qs],
                                 start=False, stop=True)
            pb_k = rpp.tile([128, 512], BF16, tag="pb", name="pb")
            nc.vector.tensor_copy(pb_k, ps_k)
            rope_b2(pb_k, kt_raw, qs, qs, ps_k)
            nc.gpsimd.dma_start(out=ktd[0][64:128, qs], in_=kt_raw[0:64, qs])
            nc.gpsimd.dma_start(out=ktd[1][0:64, qs], in_=kt_raw[64:128, qs])
            pb_q = rpp.tile([128, 512], BF16, tag="pb", name="pb")
            nc.vector.tensor_copy(pb_q, ps_q)
            rope_b2(pb_q, qt[0], qs, qs, ps_q)

        kq_startup()

        def pop_pv_fillers():
            pq, ph, ppts = pending.popleft()
            pvt = pjp.tile([128, 512], F32, tag="pj", name="pj")
            segs = [lambda t=t: pv_segment(pq, ph, ppts, pvt, t) for t in range(4)]

            def last():
                pv_segment(pq, ph, ppts, pvt, 3)
                pv_finalize(pq, ph, pvt)
            segs[3] = last
            return segs

        for q in range(NQC):
            for pos in range(NHL):
                h = HORDER[pos]
                if pos == 0:
                    at_tiles[q] = [atp.tile([128, 512], BF16, tag="at", name="at")
                                   for _ in range(4)]
                if (q, h // 2) not in anp_of:
                    anp_of[(q, h // 2)] = anpool.tile([128, 512], BF16,
                                                      tag="anp", name="anp")

                fillers = deque()
                depth = DEPTH_OF[q]
                if q == NQC - 1 and pos >= 7:
                    depth = 1
                # chunk-0 V-proj must precede the PV segments that read vt
                if q == 0:
                    if pos == 1:
                        fillers.extend([lambda: emit_vproj(0, 3),
                                        lambda: emit_vproj(3, 6)])
                    elif pos == 2:
                        fillers.extend([lambda: emit_vproj(6, 9),
                                        lambda: emit_vproj(9, 12)])
                    elif pos == 3:
                        fillers.append(lambda: emit_vproj(12, 16))
                # PV of popped heads first: their recip/norm DVE work then
                # drains mid-pos, keeping the DVE free for the trailing
                # fast-exp groups at the pos boundary
                while len(pending) > depth - 1:
                    fillers.extend(pop_pv_fillers())
                # O-proj of previous chunk (needs all its pairs transposed)
                if q > 0 and 2 <= pos <= 5:
                    for gg in (2 * (pos - 2), 2 * (pos - 2) + 1):
                        fillers.append(lambda q=q, gg=gg: oproj_group(q - 1, gg))
                # Q-proj: emitted late in the pos (its DVE rope tail is long)
                if q == 0:
                    if pos == 1:
                        fillers.append(lambda: qproj_group(0, 2))
                    elif pos == 2:
                        fillers.append(lambda: qproj_group(0, 3))
                    elif pos == 4:
                        fillers.append(lambda: qproj_group(1, 0))
                # q-proj of chunk q+1: pairs 2/3 spill past the chunk
                # boundary (chunk 0 is the most PE-loaded, chunk 3 the least)
                if q < NQC - 1:
                    if q > 0 and pos == 2:
                        fillers.append(lambda q=q: qproj_group(q + 1, 0))
                    elif pos == 5:
                        fillers.append(lambda q=q: qproj_group(q + 1, 1))
                if q > 0 and pos in (0, 1):
                    fillers.appendleft(lambda q=q, i=pos + 2: qproj_group(q, i))

                # emit scores groups with fillers interleaved; the two
                # DVE fast-exp groups go first so an idle DVE frees their
                # scp slots long before the next pos needs them (at chunk-0
                # pos 0 only k-tiles 0..3 exist yet, so DVE takes those)
                order, dve_set = list(range(NGRP)), {4, 5}
                pts = [None] * NGRP
                for gi in order:
                    pts[gi] = ptp.tile([128, KTG, 512], BF16, tag="pt", name="pt")
                for emi, gi in enumerate(order):
                    emit_score_group(q, h, gi, pts, gi in dve_set)
                    if q == 0 and pos == 0:
                        if emi == 1:
                            kproj_chunk(1)
                        elif emi == 3:
                            kproj_chunk(2)
                        elif emi == 5:
                            kproj_chunk(3)
                    elif fillers:
                        fillers.popleft()()
                if q == 0 and pos == 0:
                    qproj_group(0, 1)
                while fillers:
                    fillers.popleft()()
                pending.append((q, h, pts))

        # drain: remaining PVs (inline transposes fire as pairs complete),
        # then last-chunk O-proj
        drain_mode[0] = True
        while pending:
            for f in pop_pv_fillers():
                f()
        for gidx in range(8):
            oproj_group(NQC - 1, gidx, drain=True)
    if hasattr(nc, "compile"):
        nc.compile()
    return nc


_NC_CACHE = {}


def _get_nc(with_bias=False):
    if with_bias not in _NC_CACHE:
        _NC_CACHE[with_bias] = build_bass(with_bias)
    return _NC_CACHE[with_bias]


def _prep_core_inputs(hs, cos, sin, Wq, bq, Wk, bk, Wv, bv, Wo, bo):
    """Build the 8 per-core input dicts (host-side shard + transpose)."""
    cosT = np.ascontiguousarray(cos.reshape(S, HD).T)          # [64, S]
    sinT = np.ascontiguousarray(sin.reshape(S, HD).T)
    cosT2 = np.tile(cosT, (2, 1)).astype(ml_dtypes.bfloat16)   # [128, S]
    ssinT = sinT.copy()
    ssinT[0:HD // 2] = -ssinT[0:HD // 2]
    ssinT2 = np.tile(ssinT, (2, 1)).astype(ml_dtypes.bfloat16)
    # rotate-half permutation (involution): swap halves of each 64-block
    rmat = np.zeros((128, 128), dtype=ml_dtypes.bfloat16)
    for blk in range(2):
        b0 = blk * 64
        for i in range(32):
            rmat[b0 + i, b0 + 32 + i] = 1.0
            rmat[b0 + 32 + i, b0 + i] = 1.0

    with_bias = bool(np.any(bq) or np.any(bk))
    in_maps = []
    for c in range(8):
        b, t = c // 2, c % 2
        xT = np.ascontiguousarray(hs[b].T).astype(ml_dtypes.bfloat16)  # [1024, S]

        # c-folded weight layouts: one DMA each, partition dim = within-c row
        wqT = np.empty((128, 8 * QD), ml_dtypes.bfloat16)
        wkT = np.empty((128, 8 * KD), ml_dtypes.bfloat16)
        wvT = np.zeros((128, 8 * VD), ml_dtypes.bfloat16)
        for cc in range(8):
            rows = slice(cc * 128, (cc + 1) * 128)
            wqT[:, cc * QD:(cc + 1) * QD] = Wq[t * QD:(t + 1) * QD, rows].T
            wkT[:, cc * KD:(cc + 1) * KD] = Wk[t * KD:(t + 1) * KD, rows].T
            for g in range(NKVL):
                src = Wv[t * KD + g * HD: t * KD + (g + 1) * HD, rows].T  # [128, 64]
                wvT[:, cc * VD + g * 65: cc * VD + g * 65 + HD] = src
        wvB = np.zeros((1, VD), ml_dtypes.bfloat16)
        for g in range(NKVL):
            wvB[0, g * 65:g * 65 + HD] = bv[t * KD + g * HD: t * KD + (g + 1) * HD]
            wvB[0, g * 65 + HD] = 1.0

        woT = np.empty((128, 4 * HID), ml_dtypes.bfloat16)
        for pp in range(4):
            rows = slice(t * QD + pp * 128, t * QD + (pp + 1) * 128)
            woT[:, pp * HID:(pp + 1) * HID] = Wo[:, rows].T
        m = dict(
            xT=xT, wqT=wqT, wkT=wkT, wvT=wvT, wvB=wvB, woT=woT,
            ident=np.eye(128, dtype=ml_dtypes.bfloat16),
            cosT2=cosT2, ssinT2=ssinT2, rmat=rmat,
        )
        if with_bias:
            m["wqB"] = bq[t * QD:(t + 1) * QD][None, :].astype(ml_dtypes.bfloat16)
            m["wkB"] = bk[t * KD:(t + 1) * KD][None, :].astype(ml_dtypes.bfloat16)
        in_maps.append(m)
    return in_maps


def kernel(hidden_states, cos, sin, Wq, bq, Wk, bk, Wv, bv, Wo, bo,
           _trace=False, _trace_kwargs=None):
    global LAST_RESULTS
    args = [np.asarray(a, dtype=np.float32) for a in
            (hidden_states, cos, sin, Wq, bq, Wk, bk, Wv, bv, Wo, bo)]
    in_maps = _prep_core_inputs(*args)
    with_bias = bool(np.any(args[4]) or np.any(args[6]))
    nc = _get_nc(with_bias)
    kw = dict(_trace_kwargs or {})
    res = run_bass_kernel_spmd(nc, in_maps, core_ids=list(range(8)),
                               trace=_trace, **kw)
    LAST_RESULTS = res
    outs = [r["out"] for r in res.results]
    full = np.empty((B, S, HID), np.float32)
    bo = args[10]
    for b in range(B):
        full[b] = (outs[2 * b].astype(np.float32)
                   + outs[2 * b + 1].astype(np.float32) + bo[None, :])
    return full


# revision 62
# speedup vs baseline: 1.0456x; 1.0456x over previous
"""BSRoformer attention kernel for 8 TRN2 NeuronCores (v3).

Sharding: 8 cores = 4 batch elements x 2 tensor-parallel halves.
Core c handles batch b = c//2, TP half t = c%2 (q heads 8t..8t+8,
kv heads 2t..2t+2, Wo columns 512t..512t+512). Host sums the two
row-parallel O-projection partials per batch element.

v3 (vs v2): the softmax exp is split across engines -- ACT exps 12 of
16 k-tiles, DVE computes k-tiles 8..11 with a bitcast fast-exp
(int16(round(x*c1+c2)) viewed as bf16 == 2^y with linear mantissa
interpolation; c2 is centered so the approximation is mean-unbiased and
softmax normalization cancels the residual ~2% ripple). Scores flow
through a 3-deep ring of 2-bank PSUM groups (the two DVE groups sit
mid-pos so the ring slots the next head needs first are freed early --
with only 2 slots, one dep-waiting score group clogs the PE's 4-deep
wait queue and stalls ready work behind it). PV accumulates all 4
q-subtiles into one bank (one strided reciprocal per head), the
anp->at transposes ride the XBAR DMA engine (one 3-D dma_transpose per
pair; PE transposes only at the drain where the DMA round-trip would
sit on the tail), each weight matrix lands in SBUF via one DMA with
the x/wq loads staged in chase order, O-projection output is bf16
(host upcasts), and next-chunk Q-projections spill past the chunk
boundary to balance chunk-0's K/V-proj load.
"""

import sys
from collections import deque
from contextlib import ExitStack

import numpy as np

try:
    import concourse.bass as bass  # noqa: F401
except Exception:  # pragma: no cover
    sys.path.insert(0, "/opt/trn_rl_repo")
    import concourse.bass as bass  # noqa: F401

import ml_dtypes
import concourse.mybir as mybir
import concourse.tile as tile
from concourse import bacc
from concourse.bass_utils import run_bass_kernel_spmd

F32 = mybir.dt.float32
BF16 = mybir.dt.bfloat16
I16 = mybir.dt.int16

B, S, HID = 4, 2048, 1024
NH, NKV, HD = 16, 4, 64
SCALE = HD ** -0.5
NHL, NKVL = 8, 2               # per-core q heads / kv heads (TP half)
QD, KD = NHL * HD, NKVL * HD   # 512, 128
VD = KD + NKVL                 # 130: [v_g0(64) | ones | v_g1(64) | ones]
NKT = S // 128                 # 16 key tiles
KTG = 2                        # key tiles per psum scores group
NGRP = NKT // KTG              # 8 score groups per (q-chunk, head)
NDVE = 2                       # trailing groups computed by DVE fast-exp
NQC = S // 512                 # 4 query chunks
ALU = mybir.AluOpType

# fast-exp: bf16 bits of 2^(x*log2e) ~= int16(x*FEXP_C1 + FEXP_C2).
# FEXP_C2 is lowered by 128*log2(E[(1+f)2^-f]) = 7.3737 so the linear
# mantissa interpolation is mean-unbiased against the exact-exp tiles.
FEXP_C1 = SCALE * 128.0 * 1.4426950408889634
FEXP_C2 = 127.0 * 128.0 - 7.3737

LAST_RESULTS = None


def build_bass(with_bias=False):
    nc = bacc.Bacc()
    inp = {}
    decls = [
        ("xT", [HID, S], BF16),
        ("wkT", [128, 8 * KD], BF16),    # c-folded: [r, c*128+col]
        ("wqT", [128, 8 * QD], BF16),    # c-folded: [r, c*512+m]
        ("wvT", [128, 8 * VD], BF16),    # c-folded: [r, c*130+col]
        ("wvB", [1, VD], BF16),          # bias row + ones columns
        ("woT", [128, 4 * HID], BF16),   # pair-folded: [r, pp*1024+col]
        ("ident", [128, 128], BF16),
        ("cosT2", [128, S], BF16),
        ("ssinT2", [128, S], BF16),
        ("rmat", [128, 128], BF16),
    ]
    if with_bias:
        decls += [("wqB", [1, QD], BF16), ("wkB", [1, KD], BF16)]
    for name, shape, dt in decls:
        inp[name] = nc.declare_dram_parameter(name, shape, dt, isOutput=False).ap()
    out = nc.declare_dram_parameter("out", [S, HID], BF16, isOutput=True).ap()

    with ExitStack() as ctx:
        tc = ctx.enter_context(tile.TileContext(nc))
        consts = ctx.enter_context(tc.tile_pool(name="consts", bufs=1))
        qtp = ctx.enter_context(tc.tile_pool(name="qtp", bufs=1))
        ktp = ctx.enter_context(tc.tile_pool(name="ktp", bufs=1))
        vp = ctx.enter_context(tc.tile_pool(name="vp", bufs=1))
        rpp = ctx.enter_context(tc.tile_pool(name="rpp", bufs=6))
        tsp = ctx.enter_context(tc.tile_pool(name="tsp", bufs=6))
        ptp = ctx.enter_context(tc.tile_pool(name="ptp", bufs=33))
        rdp = ctx.enter_context(tc.tile_pool(name="rdp", bufs=8))
        anpool = ctx.enter_context(tc.tile_pool(name="anpool", bufs=8))
        atp = ctx.enter_context(tc.tile_pool(name="atp", bufs=8))
        osp = ctx.enter_context(tc.tile_pool(name="osp", bufs=6))
        pjp = ctx.enter_context(tc.tile_pool(name="pjp", bufs=2, space="PSUM"))
        scp = ctx.enter_context(tc.tile_pool(name="scp", bufs=3, space="PSUM"))

        # ---------------- constants / weights in SBUF ----------------
        # One DMA per weight matrix (c-folded host layouts); x arrives in
        # 4 column phases (phase 0 per-c) so K-proj chases the loads.
        wk_sb = consts.tile([128, 8 * KD], BF16, tag="wk", name="wk")
        nc.sync.dma_start(out=wk_sb, in_=inp["wkT"])
        xt_all = consts.tile([128, 8 * S], BF16, tag="xt", name="xt")
        wq_sb = consts.tile([128, 8 * QD], BF16, tag="wq", name="wq")
        wq_v = wq_sb[:, :].rearrange("p (c m) -> p c m", c=8)
        wqT_v = inp["wqT"][:, :].rearrange("p (c m) -> p c m", c=8)

        def xt(c, lo, hi):
            return xt_all[:, c * S + lo: c * S + hi]

        xt_v = xt_all[:, :].rearrange("p (c s) -> p c s", c=8)
        xT_v = inp["xT"][:, :].rearrange("(c p) s -> p c s", c=8)

        def dma_x_phase(ph, halves=False):
            sl = slice(ph * 512, (ph + 1) * 512)
            if halves:
                nc.sync.dma_start(out=xt_v[:, 0:4, sl], in_=xT_v[:, 0:4, sl])
                nc.sync.dma_start(out=xt_v[:, 4:8, sl], in_=xT_v[:, 4:8, sl])
            else:
                nc.sync.dma_start(out=xt_v[:, :, sl], in_=xT_v[:, :, sl])

        # startup critical path: x phase-0 per-c right after wk so the fused
        # K/Q-proj chases the DMAs; the wq pair-0 slice lands right after
        # x c0 (the fused loop needs it at c0), rope tables in halves, and
        # the x phases run ahead of the bulk weight loads
        def dma_xc(c):
            nc.sync.dma_start(out=xt_all[:, c * S: c * S + 512],
                              in_=inp["xT"][c * 128:(c + 1) * 128, 0:512])

        dma_xc(0)
        nc.sync.dma_start(out=wq_v[:, :, 0:128], in_=wqT_v[:, :, 0:128])
        for c in range(1, 4):
            dma_xc(c)
        rmat = consts.tile([128, 128], BF16, tag="rmat", name="rmat")
        nc.sync.dma_start(out=rmat, in_=inp["rmat"])
        cos_sb = consts.tile([128, S], BF16, tag="cos", name="cos")
        sin_sb = consts.tile([128, S], BF16, tag="sin", name="sin")
        nc.sync.dma_start(out=cos_sb[:, 0:512], in_=inp["cosT2"][:, 0:512])
        nc.sync.dma_start(out=sin_sb[:, 0:512], in_=inp["ssinT2"][:, 0:512])
        for c in range(4, 8):
            dma_xc(c)
        dma_x_phase(1, halves=True)
        nc.sync.dma_start(out=wq_v[:, :, 128:256], in_=wqT_v[:, :, 128:256])
        nc.sync.dma_start(out=cos_sb[:, 512:2048], in_=inp["cosT2"][:, 512:2048])
        nc.sync.dma_start(out=sin_sb[:, 512:2048], in_=inp["ssinT2"][:, 512:2048])
        wv_b = consts.tile([1, VD], BF16, tag="wv_b", name="wv_b")
        nc.sync.dma_start(out=wv_b, in_=inp["wvB"])
        dma_x_phase(2, halves=True)
        nc.sync.dma_start(out=wq_v[:, :, 256:512], in_=wqT_v[:, :, 256:512])
        wv_sb = consts.tile([128, 8 * VD], BF16, tag="wv", name="wv")
        nc.sync.dma_start(out=wv_sb, in_=inp["wvT"])
        dma_x_phase(3)
        wo_sb = consts.tile([128, 4 * HID], BF16, tag="wo", name="wo")
        nc.sync.dma_start(out=wo_sb, in_=inp["woT"])
        identT = consts.tile([128, 128], BF16, tag="ident", name="ident")
        nc.sync.dma_start(out=identT, in_=inp["ident"])
        if with_bias:
            wq_b = consts.tile([1, QD], BF16, tag="wq_b", name="wq_b")
            nc.sync.dma_start(out=wq_b, in_=inp["wqB"])
            wk_b = consts.tile([1, KD], BF16, tag="wk_b", name="wk_b")
            nc.sync.dma_start(out=wk_b, in_=inp["wkB"])
        ones_f = consts.tile([1, S], BF16, tag="ones_f", name="ones_f")
        nc.vector.memset(ones_f, 1.0)

        # ---------------- rope ----------------
        # rotate-half runs on the PE (rot_ps = P_swap @ pb); the DVE ops are
        # then all partition-aligned: ts = rot_ps*sgn-sin, dst = pb*cos + ts.
        # rot_ps reuses the projection's own psum bank (its data is already
        # evacuated to pb when the rotate matmul runs).
        def rope_b2(pb, dst, dsl, qs, rot_ps):
            nc.tensor.matmul(rot_ps, rmat, pb, start=True, stop=True)
            ts = tsp.tile([128, 512], BF16, tag="ts", name="ts")
            nc.vector.tensor_mul(ts, rot_ps, sin_sb[:, qs])
            nc.vector.tensor_mul(dst[:, dsl], pb, cos_sb[:, qs])
            nc.vector.tensor_add(dst[:, dsl], dst[:, dsl], ts)

        # ---------------- K projection + rope + head-pair dup ----------------
        kt_raw = ktp.tile([128, S], BF16, tag="kt_raw", name="kt_raw")
        ktd = [ktp.tile([128, S], BF16, tag=f"ktd{g}", name=f"ktd{g}") for g in range(2)]

        def kproj_chunk(q4):
            qs = slice(q4 * 512, (q4 + 1) * 512)
            ps = pjp.tile([128, 512], F32, tag="pj", name="pj")
            for c in range(8):
                nc.tensor.matmul(ps, wk_sb[:, c * KD:(c + 1) * KD],
                                 xt(c, qs.start, qs.stop), start=(c == 0),
                                 stop=(not with_bias and c == 7))
            if with_bias:
                nc.tensor.matmul(ps, wk_b, ones_f[0:1, qs], start=False, stop=True)
            pb = rpp.tile([128, 512], BF16, tag="pb", name="pb")
            nc.vector.tensor_copy(pb, ps)
            rope_b2(pb, kt_raw, qs, qs, ps)
            nc.gpsimd.dma_start(out=ktd[0][64:128, qs], in_=kt_raw[0:64, qs])
            nc.gpsimd.dma_start(out=ktd[1][0:64, qs], in_=kt_raw[64:128, qs])

        # ---------------- Q projection (per chunk, pipelined) ----------------
        qt = [qtp.tile([128, 1024], BF16, tag=f"qt{i}", name=f"qt{i}") for i in range(4)]

        def qproj_group(qc, i):
            qs = slice(qc * 512, (qc + 1) * 512)
            slot = slice((qc % 2) * 512, (qc % 2 + 1) * 512)
            ps = pjp.tile([128, 512], F32, tag="pj", name="pj")
            for c in range(8):
                nc.tensor.matmul(ps, wq_sb[:, c * QD + i * 128: c * QD + (i + 1) * 128],
                                 xt(c, qs.start, qs.stop), start=(c == 0),
                                 stop=(not with_bias and c == 7))
            if with_bias:
                nc.tensor.matmul(ps, wq_b[:, i * 128:(i + 1) * 128],
                                 ones_f[0:1, qs], start=False, stop=True)
            pb = rpp.tile([128, 512], BF16, tag="pb", name="pb")
            nc.vector.tensor_copy(pb, ps)
            rope_b2(pb, qt[i], slot, qs, ps)

        # ---------------- V projection (natural, with ones columns) --------
        vt = [vp.tile([128, VD], BF16, tag=f"v{i}", name=f"v{i}") for i in range(NKT)]

        def emit_vproj(i0, i1):
            for i in range(i0, i1):
                tsl = slice(i * 128, (i + 1) * 128)
                ps = pjp.tile([128, 512], F32, tag="pj", name="pj")
                psv = ps[:, 0:VD]
                for c in range(8):
                    nc.tensor.matmul(psv, xt(c, tsl.start, tsl.stop),
                                     wv_sb[:, c * VD:(c + 1) * VD],
                                     start=(c == 0), stop=False)
                # the wv bias row also carries the ones columns that produce
                # the softmax denominators, so it is always emitted
                nc.tensor.matmul(psv, ones_f[0:1, tsl], wv_b, start=False, stop=True)
                nc.vector.tensor_copy(vt[i], psv)

        # ---------------- attention helpers ----------------
        at_tiles = {}
        anp_of = {}

        def emit_score_group(q, h, gi, pts, on_dve):
            p, ab, g = h // 2, h % 2, h // 4
            qs = slice((q % 2) * 512, (q % 2 + 1) * 512)
            qrow = slice(ab * 64, (ab + 1) * 64)
            kt_src = kt_raw if ab == g else ktd[g]
            sc = scp.tile([128, KTG * 512], F32, tag="sc", name="sc")
            for j in range(KTG):
                kt_i = gi * KTG + j
                ksl = slice(kt_i * 128, (kt_i + 1) * 128)
                nc.tensor.matmul(sc[:, j * 512:(j + 1) * 512],
                                 kt_src[qrow, ksl], qt[p][qrow, qs],
                                 start=True, stop=True)
            pt = pts[gi]
            if on_dve:
                nc.vector.tensor_scalar(pt.bitcast(I16), sc, FEXP_C1, FEXP_C2,
                                        ALU.mult, ALU.add)
            else:
                nc.scalar.activation(out=pt, in_=sc,
                                     func=mybir.ActivationFunctionType.Exp,
                                     scale=SCALE)

        def pv_segment(q, h, pts, pvt, t):
            g = h // 4
            dst = pvt[:, t * 65:(t + 1) * 65]
            for gi in range(NGRP):
                for j in range(KTG):
                    kt_i = gi * KTG + j
                    nc.tensor.matmul(dst, pts[gi][:, j, t * 128:(t + 1) * 128],
                                     vt[kt_i][:, g * 65:(g + 1) * 65],
                                     start=(kt_i == 0), stop=(kt_i == NKT - 1))

        pair_done = {}

        def pv_finalize(q, h, pvt):
            p, ab = h // 2, h % 2
            anp = anp_of[(q, p)]
            rd = rdp.tile([128, 4], F32, tag="rd", name="rd")
            nc.vector.reciprocal(rd, pvt[:, 64:64 + 4 * 65:65])
            for t in range(4):
                base = t * 128 + ab * 64
                nc.vector.tensor_scalar(anp[:, base:base + 64],
                                        pvt[:, t * 65:t * 65 + 64],
                                        rd[:, t:t + 1], None, ALU.mult)
            n = pair_done.get((q, p), 0) + 1
            pair_done[(q, p)] = n
            if n == 2:
                transpose_pair(q, p, range(4), use_pe=drain_mode[0])

        def transpose_pair(q, p, ts_, use_pe=False):
            # one XBAR DMA transposes the whole pair: out[d, t, q] =
            # anp[q, t*128+d] (the per-q-subtile transpose the O-proj
            # needs). The drain uses PE transposes instead: the ~2.8us DMA
            # round-trip would sit on the critical tail path.
            at = at_tiles[q]
            anp = anp_of[(q, p)]
            if use_pe:
                tp_f = pjp.tile([128, 512], F32, tag="pj", name="pj")
                tp_b = tp_f.bitcast(BF16)
                for t in ts_:
                    nc.tensor.transpose(tp_b[:, t * 128:(t + 1) * 128],
                                        anp[:, t * 128:(t + 1) * 128], identT)
                nc.vector.tensor_copy(at[p][:, 0:512], tp_b[:, 0:512])
            else:
                out_v = at[p][:, 0:512].rearrange("d (t q) -> d t q", t=4)
                nc.sync.dma_start_transpose(out=out_v, in_=anp[:, 0:512])

        def oproj_group(q, gidx, drain=False):
            t_, nh = gidx // 2, gidx % 2
            at = at_tiles[q]
            rows = slice(q * 512 + t_ * 128, q * 512 + (t_ + 1) * 128)
            tsl = slice(t_ * 128, (t_ + 1) * 128)
            nsl = slice(nh * 512, (nh + 1) * 512)
            if drain and gidx % 2 == 1:
                # tail: scores banks are free, use them to widen the pipeline
                po = scp.tile([128, KTG * 512], F32, tag="sc", name="sc")[:, 0:512]
            else:
                po = pjp.tile([128, 512], F32, tag="pj", name="pj")
            for pp in range(4):
                nc.tensor.matmul(po, at[pp][:, tsl],
                                 wo_sb[:, pp * HID + nsl.start: pp * HID + nsl.stop],
                                 start=(pp == 0), stop=(pp == 3))
            os_ = osp.tile([128, 512], BF16, tag="os", name="os")
            if drain and gidx % 2 == 0:
                nc.scalar.activation(out=os_, in_=po,
                                     func=mybir.ActivationFunctionType.Copy)
            else:
                nc.vector.tensor_copy(os_, po)
            nc.sync.dma_start(out=out[rows, nsl], in_=os_)

        # ---------------- emission schedule ----------------
        # Scores of head (q,pos) are interleaved with "filler" PE segments
        # (PV of the head popped from the pipeline, O-proj of the previous
        # chunk, Q-proj of the next, chunk-0 K/V-proj) so the PE stream
        # always has work while ACT/DVE chew on exps.
        HORDER = [0, 2, 5, 7, 1, 3, 4, 6]  # dup-free heads (ab==g) first
        pending = deque()
        drain_mode = [False]
        DEPTH_OF = [3, 2, 2, 2]

        # fused K-proj(chunk0) + Q-proj(chunk0, pair0): the two accumulation
        # series interleave per-c so both chase the per-c x DMAs
        def kq_startup():
            qs = slice(0, 512)
            ps_k = pjp.tile([128, 512], F32, tag="pj", name="pj")
            ps_q = pjp.tile([128, 512], F32, tag="pj", name="pj")
            last = not with_bias
            for c in range(8):
                nc.tensor.matmul(ps_k, wk_sb[:, c * KD:(c + 1) * KD],
                                 xt(c, 0, 512), start=(c == 0),
                                 stop=(last and c == 7))
                nc.tensor.matmul(ps_q, wq_sb[:, c * QD:c * QD + 128],
                                 xt(c, 0, 512), start=(c == 0),
                                 stop=(last and c == 7))
            if with_bias:
                nc.tensor.matmul(ps_k, wk_b, ones_f[0:1, qs], start=False, stop=True)
                nc.tensor.matmul(ps_q, wq_b[:, 0:128], ones_f[0:1, qs],
                                 start=False, stop=True)
            pb_k = rpp.tile([128, 512], BF16, tag="pb", name="pb")
            nc.vector.tensor_copy(pb_k, ps_k)
            rope_b2(pb_k, kt_raw, qs, qs, ps_k)
            nc.gpsimd.dma_start(out=ktd[0][64:128, qs], in_=kt_raw[0:64, qs])
            nc.gpsimd.dma_start(out=ktd[1][0:64, qs], in_=kt_raw[64:128, qs])
            pb_q = rpp.tile([128, 512], BF16, tag="pb", name="pb")
            nc.vector.tensor_copy(pb_q, ps_q)
            rope_b2(pb_q, qt[0], qs, qs, ps_q)

        kq_startup()

        def pop_pv_fillers():
            # six segments: PV t=0..3 (+reciprocal with t=3), then the four
            # normalize multiplies split 2+2 so the DVE reaches the g4/g5
            # fast-exps with at most ~130ns of finalize work queued ahead
            pq, ph, ppts = pending.popleft()
            p, ab = ph // 2, ph % 2
            pvt = pjp.tile([128, 512], F32, tag="pj", name="pj")
            rd = rdp.tile([128, 4], F32, tag="rd", name="rd")
            segs = [lambda t=t: pv_segment(pq, ph, ppts, pvt, t) for t in range(3)]

            def seg3():
                pv_segment(pq, ph, ppts, pvt, 3)
                nc.vector.reciprocal(rd, pvt[:, 64:64 + 4 * 65:65])

            def norm(ts_, fin):
                anp = anp_of[(pq, p)]
                for t in ts_:
                    base = t * 128 + ab * 64
                    nc.vector.tensor_scalar(anp[:, base:base + 64],
                                            pvt[:, t * 65:t * 65 + 64],
                                            rd[:, t:t + 1], None, ALU.mult)
                if fin:
                    n = pair_done.get((pq, p), 0) + 1
                    pair_done[(pq, p)] = n
                    if n == 2:
                        transpose_pair(pq, p, range(4), use_pe=drain_mode[0])
            segs += [seg3, lambda: norm((0, 1), False),
                     lambda: norm((2, 3), True)]
            return segs

        for q in range(NQC):
            for pos in range(NHL):
                h = HORDER[pos]
                if pos == 0:
                    at_tiles[q] = [atp.tile([128, 512], BF16, tag="at", name="at")
                                   for _ in range(4)]
                if (q, h // 2) not in anp_of:
                    anp_of[(q, h // 2)] = anpool.tile([128, 512], BF16,
                                                      tag="anp", name="anp")

                fillers = deque()
                depth = DEPTH_OF[q]
                if q == NQC - 1 and pos >= 7:
                    depth = 1
                # chunk-0 V-proj must precede the PV segments that read vt
                if q == 0:
                    if pos == 1:
                        fillers.extend([lambda: emit_vproj(0, 2),
                                        lambda: emit_vproj(2, 4),
                                        lambda: emit_vproj(4, 6)])
                    elif pos == 2:
                        fillers.extend([lambda: emit_vproj(6, 8),
                                        lambda: emit_vproj(8, 10),
                                        lambda: emit_vproj(10, 12)])
                    elif pos == 3:
                        fillers.extend([lambda: emit_vproj(12, 14),
                                        lambda: emit_vproj(14, 16)])
                # PV of popped heads first: their recip/norm DVE work then
                # drains mid-pos, keeping the DVE free for the trailing
                # fast-exp groups at the pos boundary
                while len(pending) > depth - 1:
                    fillers.extend(pop_pv_fillers())
                # O-proj of previous chunk (needs all its pairs transposed)
                if q > 0 and 3 <= pos <= 6:
                    for gg in (2 * (pos - 3), 2 * (pos - 3) + 1):
                        fillers.append(lambda q=q, gg=gg: oproj_group(q - 1, gg))
                # Q-proj: emitted late in the pos (its DVE rope tail is long)
                if q == 0 and pos == 3:
                    fillers.append(lambda: qproj_group(1, 0))
                # q-proj of chunk q+1: pairs 2/3 spill past the chunk
                # boundary (chunk 0 is the most PE-loaded, chunk 3 the least)
                if q < NQC - 1:
                    if q > 0 and pos == 3:
                        fillers.append(lambda q=q: qproj_group(q + 1, 0))
                    elif pos == 6:
                        fillers.append(lambda q=q: qproj_group(q + 1, 1))
                if q > 0 and pos in (1, 2):
                    fillers.appendleft(lambda q=q, i=pos + 1: qproj_group(q, i))

                # emit scores groups with fillers interleaved; the two
                # DVE fast-exp groups go first so an idle DVE frees their
                # scp slots long before the next pos needs them (at chunk-0
                # pos 0 only k-tiles 0..3 exist yet, so DVE takes those)
                order, dve_set = list(range(NGRP)), {2, 6}
                pts = [None] * NGRP
                for gi in order:
                    pts[gi] = ptp.tile([128, KTG, 512], BF16, tag="pt", name="pt")
                for emi, gi in enumerate(order):
                    emit_score_group(q, h, gi, pts, gi in dve_set)
                    if q == 0 and pos == 0:
                        if emi == 1:
                            kproj_chunk(1)
                            qproj_group(0, 1)
                        elif emi == 3:
                            kproj_chunk(2)
                            qproj_group(0, 2)
                        elif emi == 5:
                            kproj_chunk(3)
                            qproj_group(0, 3)
                    elif fillers:
                        fillers.popleft()()
                while fillers:
                    fillers.popleft()()
                pending.append((q, h, pts))

        # drain: remaining PVs (inline transposes fire as pairs complete),
        # then last-chunk O-proj
        drain_mode[0] = True
        while pending:
            for f in pop_pv_fillers():
                f()
        for gidx in range(8):
            oproj_group(NQC - 1, gidx, drain=True)
    if hasattr(nc, "compile"):
        nc.compile()
    return nc


_NC_CACHE = {}


def _get_nc(with_bias=False):
    if with_bias not in _NC_CACHE:
        _NC_CACHE[with_bias] = build_bass(with_bias)
    return _NC_CACHE[with_bias]


def _prep_core_inputs(hs, cos, sin, Wq, bq, Wk, bk, Wv, bv, Wo, bo):
    """Build the 8 per-core input dicts (host-side shard + transpose)."""
    cosT = np.ascontiguousarray(cos.reshape(S, HD).T)          # [64, S]
    sinT = np.ascontiguousarray(sin.reshape(S, HD).T)
    cosT2 = np.tile(cosT, (2, 1)).astype(ml_dtypes.bfloat16)   # [128, S]
    ssinT = sinT.copy()
    ssinT[0:HD // 2] = -ssinT[0:HD // 2]
    ssinT2 = np.tile(ssinT, (2, 1)).astype(ml_dtypes.bfloat16)
    # rotate-half permutation (involution): swap halves of each 64-block
    rmat = np.zeros((128, 128), dtype=ml_dtypes.bfloat16)
    for blk in range(2):
        b0 = blk * 64
        for i in range(32):
            rmat[b0 + i, b0 + 32 + i] = 1.0
            rmat[b0 + 32 + i, b0 + i] = 1.0

    with_bias = bool(np.any(bq) or np.any(bk))
    in_maps = []
    for c in range(8):
        b, t = c // 2, c % 2
        xT = np.ascontiguousarray(hs[b].T).astype(ml_dtypes.bfloat16)  # [1024, S]

        # c-folded weight layouts: one DMA each, partition dim = within-c row
        wqT = np.empty((128, 8 * QD), ml_dtypes.bfloat16)
        wkT = np.empty((128, 8 * KD), ml_dtypes.bfloat16)
        wvT = np.zeros((128, 8 * VD), ml_dtypes.bfloat16)
        for cc in range(8):
            rows = slice(cc * 128, (cc + 1) * 128)
            wqT[:, cc * QD:(cc + 1) * QD] = Wq[t * QD:(t + 1) * QD, rows].T
            wkT[:, cc * KD:(cc + 1) * KD] = Wk[t * KD:(t + 1) * KD, rows].T
            for g in range(NKVL):
                src = Wv[t * KD + g * HD: t * KD + (g + 1) * HD, rows].T  # [128, 64]
                wvT[:, cc * VD + g * 65: cc * VD + g * 65 + HD] = src
        wvB = np.zeros((1, VD), ml_dtypes.bfloat16)
        for g in range(NKVL):
            wvB[0, g * 65:g * 65 + HD] = bv[t * KD + g * HD: t * KD + (g + 1) * HD]
            wvB[0, g * 65 + HD] = 1.0

        woT = np.empty((128, 4 * HID), ml_dtypes.bfloat16)
        for pp in range(4):
            rows = slice(t * QD + pp * 128, t * QD + (pp + 1) * 128)
            woT[:, pp * HID:(pp + 1) * HID] = Wo[:, rows].T
        m = dict(
            xT=xT, wqT=wqT, wkT=wkT, wvT=wvT, wvB=wvB, woT=woT,
            ident=np.eye(128, dtype=ml_dtypes.bfloat16),
            cosT2=cosT2, ssinT2=ssinT2, rmat=rmat,
        )
        if with_bias:
            m["wqB"] = bq[t * QD:(t + 1) * QD][None, :].astype(ml_dtypes.bfloat16)
            m["wkB"] = bk[t * KD:(t + 1) * KD][None, :].astype(ml_dtypes.bfloat16)
        in_maps.append(m)
    return in_maps


def kernel(hidden_states, cos, sin, Wq, bq, Wk, bk, Wv, bv, Wo, bo,
           _trace=False, _trace_kwargs=None):
    global LAST_RESULTS
    args = [np.asarray(a, dtype=np.float32) for a in
            (hidden_states, cos, sin, Wq, bq, Wk, bk, Wv, bv, Wo, bo)]
    in_maps = _prep_core_inputs(*args)
    with_bias = bool(np.any(args[4]) or np.any(args[6]))
    nc = _get_nc(with_bias)
    kw = dict(_trace_kwargs or {})
    res = run_bass_kernel_spmd(nc, in_maps, core_ids=list(range(8)),
                               trace=_trace, **kw)
    LAST_RESULTS = res
    outs = [r["out"] for r in res.results]
    full = np.empty((B, S, HID), np.float32)
    bo = args[10]
    for b in range(B):
        full[b] = (outs[2 * b].astype(np.float32)
                   + outs[2 * b + 1].astype(np.float32) + bo[None, :])
    return full
